# revision 1
# baseline (speedup 1.0000x reference)
"""Trainium2 Bass kernel for nn_CorrelationLoss.

reference math:
    G[i, j] = D[idx_i, idx_j]
    total   = sum_{i > j} G[i, j] * w[i - 1 - j]
    correlation_loss = -total / (n - 1)
    weight_loss      = -5 * sum(min(w[:-1] - w[1:], 0)) = 5 * sum(relu(w[1:] - w[:-1]))

Distribution: shard i across the 8 cores, triangle-balanced: core m owns the
8 row blocks i in [1024k + 128m, 1024k + 128m + 128), k = 0..7.  D is
replicated.  Per block:
  1. SWDGE indirect DMA gathers the 128 rows D[idx_i, :] -> R [128, 8192]
  2. gpsimd ap_gather gathers columns: G[p, j] = R[p, idx_j], j < 1024(k+1)
  3. DVE tensor_tensor_reduce: acc[p] += sum_j G[p, j] * W[p, j]
W (the triangular Toeplitz weight image) depends only on i - j, so one
[128, 8192] SBUF image covers every block: W_sb[p, 128v + jj] read from a
per-core zero-padded, shifted copy of w via a skewed (negative-stride) DMA;
the zero padding implements both the j >= i triangle and per-core row offset.
"""

import numpy as np

N = 8192          # n indices
CB = 8192         # codebook size
P = 128           # partitions
NCORES = 8
KBLK = 8          # row blocks per core
WBUF_LEN = 9344   # 1024-128m zeros | w (8191) | tail zeros
WLIN_LEN = 8320   # w (8191) | 129 zeros


def _build_nc():
    import concourse.bacc as bacc
    import concourse.mybir as mybir
    import concourse.tile as tile
    from concourse.bass import AP, IndirectOffsetOnAxis

    f32 = mybir.dt.float32
    i32 = mybir.dt.int32
    i16 = mybir.dt.int16

    nc = bacc.Bacc("TRN2", target_bir_lowering=False, debug=False)

    d_mat = nc.dram_tensor("d_mat", [N, CB], f32, kind="ExternalInput")
    idxw = nc.dram_tensor("idxw", [P, N // 16], i16, kind="ExternalInput")
    rowidx = nc.dram_tensor("rowidx", [P, KBLK], i32, kind="ExternalInput")
    wbuf = nc.dram_tensor("wbuf", [WBUF_LEN], f32, kind="ExternalInput")
    wlin = nc.dram_tensor("wlin", [WLIN_LEN], f32, kind="ExternalInput")
    out = nc.dram_tensor("out", [P, 2], f32, kind="ExternalOutput")

    with tile.TileContext(nc) as tc:
        with (
            tc.tile_pool(name="persist", bufs=1) as pp,
            tc.tile_pool(name="rpool", bufs=2) as rp,
            tc.tile_pool(name="gpool", bufs=2) as gp,
        ):
            # --- persistent tiles ---
            # flipped W image: w_flip[p, c] = wbuf[p + c]; block k reads it
            # reversed so that element (p, j) = wbuf[1023 + 1024k + p - j]
            w_flip = pp.tile([P, 64 * P], f32, tag="w_flip")
            idxw_sb = pp.tile([P, N // 16], i16, tag="idxw_sb")
            rowidx_sb = pp.tile([P, KBLK], i32, tag="rowidx_sb")
            part = [
                pp.tile([P, 1], f32, tag=f"part{k}", name=f"part{k}")
                for k in range(KBLK)
            ]
            wl_t = pp.tile([P, 65], f32, tag="wl_t")
            wl_d = pp.tile([P, 64], f32, tag="wl_d")
            wl_acc = pp.tile([P, 1], f32, tag="wl_acc")

            # --- small input loads ---
            nc.sync.dma_start(idxw_sb[:], idxw[:, :])
            nc.sync.dma_start(rowidx_sb[:], rowidx[:, :])

            # W image (positive strides only; HWDGE rejects negative steps)
            w_src = AP(wbuf, 0, [[1, P], [1, 64 * P]])
            nc.sync.dma_start(w_flip[:], w_src)

            # weight-regularization term: 5 * sum(relu(w[t+1] - w[t]))
            wl_src = AP(wlin, 0, [[64, P], [1, 65]])
            nc.sync.dma_start(wl_t[:], wl_src)
            nc.vector.tensor_tensor(
                out=wl_d[:], in0=wl_t[:, 1:65], in1=wl_t[:, 0:64],
                op=mybir.AluOpType.subtract,
            )
            nc.vector.tensor_scalar_max(wl_d[:], wl_d[:], 0.0)
            nc.vector.tensor_reduce(
                out=wl_acc[:], in_=wl_d[:],
                axis=mybir.AxisListType.X, op=mybir.AluOpType.add,
            )
            nc.sync.dma_start(out[:, 1:2], wl_acc[:])

            # --- main loop over the 8 row blocks ---
            for k in range(KBLK):
                jext = 1024 * (k + 1)          # gathered column extent
                r_t = rp.tile([P, CB], f32, tag="r_t")
                g_t = gp.tile([P, CB], f32, tag="g_t")

                # row gather: R[p, :] = D[rowidx[p, k], :]
                nc.gpsimd.indirect_dma_start(
                    out=r_t[:],
                    out_offset=None,
                    in_=d_mat[:, :],
                    in_offset=IndirectOffsetOnAxis(
                        ap=rowidx_sb[:, k : k + 1], axis=0
                    ),
                )

                # column gather: G[p, j] = R[p, idx_j]
                nc.gpsimd.ap_gather(
                    out_ap=g_t[:, 0:jext],
                    in_ap=r_t[:],
                    idxs_ap=idxw_sb[:, 0 : jext // 16],
                    channels=P,
                    num_elems=CB,
                    d=1,
                    num_idxs=jext,
                )

                # part_k[p] = sum_j G[p, j] * W[p, j],
                # W[p, j] = w_flip[p, 1023 + 1024k - j]
                wrev = AP(
                    w_flip.tensor,
                    w_flip.offset + 1023 + 1024 * k,
                    [list(w_flip.ap[0]), [-1, jext]],
                )
                nc.vector.scalar_tensor_tensor(
                    out=g_t[:, 0:jext],
                    in0=g_t[:, 0:jext],
                    scalar=1.0,
                    in1=wrev,
                    op0=mybir.AluOpType.mult,
                    op1=mybir.AluOpType.mult,
                    accum_out=part[k][:],
                )

            for k in range(1, KBLK):
                nc.vector.tensor_tensor(
                    out=part[k][:], in0=part[k][:], in1=part[k - 1][:],
                    op=mybir.AluOpType.add,
                )
            nc.sync.dma_start(out[:, 0:1], part[KBLK - 1][:])

    nc.compile()
    return nc


def _prep_in_maps(index_ls, weight_tensor, distance_matrixA):
    idx = np.asarray(index_ls).astype(np.int64)
    w = np.asarray(weight_tensor, dtype=np.float32).reshape(-1)   # [8191]
    d = np.ascontiguousarray(np.asarray(distance_matrixA, dtype=np.float32))

    idx16 = idx.astype(np.int16)
    idxw = np.ascontiguousarray(np.tile(idx16.reshape(N // 16, 16).T, (8, 1)))
    karr = idx.astype(np.int32).reshape(KBLK, NCORES, P)  # [k, m, p]
    wlin = np.zeros(WLIN_LEN, np.float32)
    wlin[: CB - 1] = w

    in_maps = []
    for m in range(NCORES):
        pre = 1024 - 128 * m
        wbuf = np.zeros(WBUF_LEN, np.float32)
        wbuf[pre : pre + CB - 1] = w
        rowidx = np.ascontiguousarray(karr[:, m, :].T)    # [p, k]
        in_maps.append(
            {
                "d_mat": d,
                "idxw": idxw,
                "rowidx": rowidx,
                "wbuf": wbuf,
                "wlin": wlin,
            }
        )
    return in_maps


def kernel(index_ls, weight_tensor, distance_matrixA, _trace=False):
    from concourse.bass_utils import run_bass_kernel_spmd

    nc = _build_nc()
    in_maps = _prep_in_maps(index_ls, weight_tensor, distance_matrixA)
    res = run_bass_kernel_spmd(
        nc, in_maps, core_ids=list(range(NCORES)), trace=_trace
    )
    if _trace:
        kernel.last_result = res

    total = np.float64(0.0)
    for m in range(NCORES):
        total += np.asarray(res.results[m]["out"])[:, 0].astype(np.float64).sum()
    corr = -(total / (N - 1))
    wl = 5.0 * np.asarray(res.results[0]["out"])[:, 1].astype(np.float64).sum()
    return np.float32(corr), np.float32(wl)

